# revision 16
# baseline (speedup 1.0000x reference)
"""Trainium2 Bass kernel for nn_CrossSectionalAttentionFusionMLP.

Math note: the reference computes attention masks fm/lm = softmax(...) over the
last axis, then applies them via einsum('bhwv,bchw->bchw', mask, feat) — which
contracts v over the mask ONLY, i.e. multiplies feat by the softmax row-sum
(== 1).  The transfer tensors therefore equal the raw features to fp32 rounding
(~1e-7 relative, verified), so the kernel computes only

    fout = W_f @ relu(bn_f(concat([frontal, frontal])))   # (B, 512, H, W)
    lout = W_l @ relu(bn_l(concat([lateral, lateral])))

with the BatchNorm folded to per-channel scale/bias on the host.

Sharding: data-parallel over B across 8 cores (4 samples each); conv weights
and folded BN constants replicated.  Per core the 1x1 conv is a K=1024 x M=512
matmul over N=4096 pixels, done as 128x128x512 PE matmuls accumulating in PSUM,
with ScalarE producing relu(x*s+t) tiles and VectorE copying PSUM->SBUF.
"""

import os
from contextlib import ExitStack

import numpy as np

import concourse.bass as bass
import concourse.mybir as mybir
import concourse.tile as tile
from concourse import bacc
from concourse.bass_utils import run_bass_kernel_spmd

N_CORES = 8
B, C, H, W = 32, 512, 32, 32
PIX = H * W                 # 1024 pixels per sample
BS = B // N_CORES           # 4 samples per core
KC = (2 * C) // 128         # 8 contraction chunks of 128
MC = C // 128               # 4 output-channel chunks of 128
NT = 512                    # moving free dim per matmul (one PSUM bank of f32)
NN = PIX // NT              # 2 pixel chunks per sample

EPS = 1e-5

F32 = mybir.dt.float32
MM_DT = mybir.dt.float16  # full-rate PE matmul + FWL; ~3e-4 rel err at K=1024

LAST_RESULTS = None  # BassKernelResults of the most recent run (for test.py)

_CACHE = {}


def _build_module():
    nc = bacc.Bacc("TRN2", target_bir_lowering=False, debug=False)

    f_d = nc.dram_tensor("f", [BS, C, PIX], MM_DT, kind="ExternalInput").ap()
    l_d = nc.dram_tensor("l", [BS, C, PIX], MM_DT, kind="ExternalInput").ap()
    wtf_d = nc.dram_tensor("wtf", [128, KC * C], MM_DT, kind="ExternalInput").ap()
    wtl_d = nc.dram_tensor("wtl", [128, KC * C], MM_DT, kind="ExternalInput").ap()
    cf_d = nc.dram_tensor("cf", [128, 16], F32, kind="ExternalInput").ap()
    cl_d = nc.dram_tensor("cl", [128, 16], F32, kind="ExternalInput").ap()
    fo_d = nc.dram_tensor("fo", [BS, C, PIX], MM_DT, kind="ExternalOutput").ap()
    lo_d = nc.dram_tensor("lo", [BS, C, PIX], MM_DT, kind="ExternalOutput").ap()

    with tile.TileContext(nc) as tc, ExitStack() as ctx:
        wpool = ctx.enter_context(tc.tile_pool(name="weights", bufs=1))
        cpool = ctx.enter_context(tc.tile_pool(name="consts", bufs=1))
        finp = ctx.enter_context(tc.tile_pool(name="finp", bufs=10))
        rhsp = ctx.enter_context(tc.tile_pool(name="rhs", bufs=12))
        psump = ctx.enter_context(tc.tile_pool(name="psum", bufs=8, space="PSUM"))
        outp = ctx.enter_context(tc.tile_pool(name="outs", bufs=6))

        # Constants + weights go on the scalar HWDGE ring (qActDynamicHW) so
        # the sync ring (qSPDynamicHW) starts streaming features immediately;
        # output stores go on the gpsimd SWDGE ring.  Three independent
        # descriptor streams over the same 16 SDMA engines.
        c_tiles = {}
        for cname, cd in (("f", cf_d), ("l", cl_d)):
            t = cpool.tile([128, 16], F32, tag=f"c{cname}")
            nc.scalar.dma_start(t[:], cd[:, :])
            c_tiles[cname] = t
        # Replicated weights: one [128, KC, C] tile per branch (W.T chunked),
        # loaded with a single 1 MiB DMA, lazily on first use.
        w_tiles = {}

        def get_w(wname):
            if wname not in w_tiles:
                wd = wtf_d if wname == "f" else wtl_d
                t = wpool.tile([128, KC, C], MM_DT, tag=f"w{wname}")
                nc.scalar.dma_start(t[:], wd.rearrange("p (k m) -> p k m", k=KC))
                w_tiles[wname] = t
            return w_tiles[wname]

        # PE warm-up: dummy matmuls on garbage SBUF during the input DMA
        # latency so HAM un-throttles (K=8/8) before the first real matmul.
        warm = cpool.tile([128, 512], MM_DT, tag="warm")
        nc.gpsimd.memset(warm[:], 0.0)
        warm_ps = psump.tile([128, NT], F32, tag="ps", name="warmps")
        for _ in range(26):
            nc.tensor.matmul(
                warm_ps[:], warm[:, 0:128], warm[:], start=True, stop=True
            )

        for b in range(BS):
            for br, feat_d, out_d in (("f", f_d, fo_d), ("l", l_d, lo_d)):
                ct = c_tiles[br]
                wt = get_w(br)
                fin = []
                for k in range(MC):
                    t = finp.tile([128, PIX], MM_DT, tag="fin", name=f"fin{b}{br}{k}")
                    nc.sync.dma_start(t[:], feat_d[b, 128 * k : 128 * (k + 1), :])
                    fin.append(t)
                # k-chunk order interleaved (chunks 0,0,1,1,...) so the first
                # matmul group only waits on the first 512 KiB feature DMA.
                korder = [0, MC, 1, MC + 1, 2, MC + 2, 3, MC + 3]
                # One full-width ACT per k-chunk: (N+352)-cycle cost amortized.
                rhs = {}
                for k8 in korder:
                    r = rhsp.tile([128, PIX], MM_DT, tag="rhs", name=f"rhs{b}{br}{k8}")
                    nc.scalar.activation(
                        r[:],
                        fin[k8 % MC][:],
                        mybir.ActivationFunctionType.Relu,
                        bias=ct[:, 8 + k8 : 9 + k8],
                        scale=ct[:, k8 : k8 + 1],
                    )
                    rhs[k8] = r
                for m in range(MC):
                    ot = outp.tile([128, PIX], MM_DT, tag="ot")
                    pss = [psump.tile([128, NT], F32, tag="ps", name=f"ps{b}{br}{m}{n}") for n in range(NN)]
                    for ki, k8 in enumerate(korder):
                        w_ap = wt[:, k8, 128 * m : 128 * (m + 1)]
                        for n in range(NN):
                            nc.tensor.matmul(
                                pss[n][:],
                                w_ap,
                                rhs[k8][:, NT * n : NT * (n + 1)],
                                start=(ki == 0),
                                stop=(ki == KC - 1),
                            )
                    for n in range(NN):
                        nc.vector.tensor_copy(ot[:, NT * n : NT * (n + 1)], pss[n][:])
                    nc.gpsimd.dma_start(out_d[b, 128 * m : 128 * (m + 1), :], ot[:])
    nc.compile()
    return nc


def _fold_bn(bn):
    g, bb, m, v = [np.asarray(x, np.float32) for x in bn]
    s = g / np.sqrt(v + EPS)
    t = bb - m * s
    return s.astype(np.float32), t.astype(np.float32)


def _pack_consts(bn):
    # [128, 16]: col j in [0,8) = scale chunk j, col 8+j = bias chunk j
    s, t = _fold_bn(bn)
    out = np.zeros((128, 16), np.float32)
    out[:, 0:8] = s.reshape(8, 128).T
    out[:, 8:16] = t.reshape(8, 128).T
    return out


def kernel(frontal_features, lateral_features, params):
    global LAST_RESULTS
    f = np.ascontiguousarray(
        np.asarray(frontal_features, np.float16).reshape(B, C, PIX)
    )
    l = np.ascontiguousarray(
        np.asarray(lateral_features, np.float16).reshape(B, C, PIX)
    )
    def pack_w(w):
        # W (C out, 2C in) -> lhsT chunks [k, p, m] -> partition-major
        # [128, KC*C] so each partition's row is contiguous in DRAM.
        wt = np.asarray(w, np.float32).T.astype(np.float16)  # [2C, C]
        return np.ascontiguousarray(
            wt.reshape(KC, 128, C).transpose(1, 0, 2).reshape(128, KC * C)
        )

    wtf = pack_w(params["fnet"]["w"])
    wtl = pack_w(params["lnet"]["w"])
    cf = _pack_consts(params["fnet"]["bn"])
    cl = _pack_consts(params["lnet"]["bn"])

    if "nc" not in _CACHE:
        _CACHE["nc"] = _build_module()
    nc = _CACHE["nc"]

    in_maps = [
        {
            "f": f[BS * i : BS * (i + 1)],
            "l": l[BS * i : BS * (i + 1)],
            "wtf": wtf,
            "wtl": wtl,
            "cf": cf,
            "cl": cl,
        }
        for i in range(N_CORES)
    ]
    res = run_bass_kernel_spmd(
        nc,
        in_maps,
        core_ids=list(range(N_CORES)),
        trace=bool(int(os.environ.get("KERNEL_TRACE", "0"))),
    )
    LAST_RESULTS = res
    fout = np.concatenate([r["fo"] for r in res.results]).astype(np.float32).reshape(B, C, H, W)
    lout = np.concatenate([r["lo"] for r in res.results]).astype(np.float32).reshape(B, C, H, W)
    return fout, lout


# revision 17
# speedup vs baseline: 1.0075x; 1.0075x over previous
"""Trainium2 Bass kernel for nn_CrossSectionalAttentionFusionMLP.

Math note: the reference computes attention masks fm/lm = softmax(...) over the
last axis, then applies them via einsum('bhwv,bchw->bchw', mask, feat) — which
contracts v over the mask ONLY, i.e. multiplies feat by the softmax row-sum
(== 1).  The transfer tensors therefore equal the raw features to fp32 rounding
(~1e-7 relative, verified), so the kernel computes only

    fout = W_f @ relu(bn_f(concat([frontal, frontal])))   # (B, 512, H, W)
    lout = W_l @ relu(bn_l(concat([lateral, lateral])))

with the BatchNorm folded to per-channel scale/bias on the host.

Sharding: data-parallel over B across 8 cores (4 samples each); conv weights
and folded BN constants replicated.  Per core the 1x1 conv is a K=1024 x M=512
matmul over N=4096 pixels, done as 128x128x512 PE matmuls accumulating in PSUM,
with ScalarE producing relu(x*s+t) tiles and VectorE copying PSUM->SBUF.
"""

import os
from contextlib import ExitStack

import numpy as np

import concourse.bass as bass
import concourse.mybir as mybir
import concourse.tile as tile
from concourse import bacc
from concourse.bass_utils import run_bass_kernel_spmd

N_CORES = 8
B, C, H, W = 32, 512, 32, 32
PIX = H * W                 # 1024 pixels per sample
BS = B // N_CORES           # 4 samples per core
KC = (2 * C) // 128         # 8 contraction chunks of 128
MC = C // 128               # 4 output-channel chunks of 128
NT = 512                    # moving free dim per matmul (one PSUM bank of f32)
NN = PIX // NT              # 2 pixel chunks per sample

EPS = 1e-5

F32 = mybir.dt.float32
MM_DT = mybir.dt.float16  # full-rate PE matmul + FWL; ~3e-4 rel err at K=1024

LAST_RESULTS = None  # BassKernelResults of the most recent run (for test.py)

_CACHE = {}


def _build_module():
    nc = bacc.Bacc("TRN2", target_bir_lowering=False, debug=False)

    f_d = nc.dram_tensor("f", [BS, C, PIX], MM_DT, kind="ExternalInput").ap()
    l_d = nc.dram_tensor("l", [BS, C, PIX], MM_DT, kind="ExternalInput").ap()
    wtf_d = nc.dram_tensor("wtf", [128, KC * C], MM_DT, kind="ExternalInput").ap()
    wtl_d = nc.dram_tensor("wtl", [128, KC * C], MM_DT, kind="ExternalInput").ap()
    cf_d = nc.dram_tensor("cf", [128, 16], F32, kind="ExternalInput").ap()
    cl_d = nc.dram_tensor("cl", [128, 16], F32, kind="ExternalInput").ap()
    fo_d = nc.dram_tensor("fo", [BS, C, PIX], MM_DT, kind="ExternalOutput").ap()
    lo_d = nc.dram_tensor("lo", [BS, C, PIX], MM_DT, kind="ExternalOutput").ap()

    with tile.TileContext(nc) as tc, ExitStack() as ctx:
        wpool = ctx.enter_context(tc.tile_pool(name="weights", bufs=1))
        cpool = ctx.enter_context(tc.tile_pool(name="consts", bufs=1))
        finp = ctx.enter_context(tc.tile_pool(name="finp", bufs=10))
        rhsp = ctx.enter_context(tc.tile_pool(name="rhs", bufs=12))
        psump = ctx.enter_context(tc.tile_pool(name="psum", bufs=8, space="PSUM"))
        outp = ctx.enter_context(tc.tile_pool(name="outs", bufs=6))

        # Constants + weights go on the scalar HWDGE ring (qActDynamicHW) so
        # the sync ring (qSPDynamicHW) starts streaming features immediately;
        # output stores go on the gpsimd SWDGE ring.  Three independent
        # descriptor streams over the same 16 SDMA engines.
        c_tiles = {}
        for cname, cd in (("f", cf_d), ("l", cl_d)):
            t = cpool.tile([128, 16], F32, tag=f"c{cname}")
            nc.scalar.dma_start(t[:], cd[:, :])
            c_tiles[cname] = t
        # Replicated weights: one [128, KC, C] tile per branch (W.T chunked),
        # loaded with a single 1 MiB DMA, lazily on first use.
        w_tiles = {}

        def get_w(wname):
            if wname not in w_tiles:
                wd = wtf_d if wname == "f" else wtl_d
                t = wpool.tile([128, KC, C], MM_DT, tag=f"w{wname}")
                nc.scalar.dma_start(t[:], wd.rearrange("p (k m) -> p k m", k=KC))
                w_tiles[wname] = t
            return w_tiles[wname]

        # PE warm-up: dummy matmuls on garbage SBUF during the input DMA
        # latency so HAM un-throttles (K=8/8) before the first real matmul.
        warm = cpool.tile([128, 512], MM_DT, tag="warm")
        nc.gpsimd.memset(warm[:], 0.0)
        warm_ps = psump.tile([128, NT], F32, tag="ps", name="warmps")
        for _ in range(36):
            nc.tensor.matmul(
                warm_ps[:], warm[:, 0:128], warm[:], start=True, stop=True
            )

        for b in range(BS):
            for br, feat_d, out_d in (("f", f_d, fo_d), ("l", l_d, lo_d)):
                ct = c_tiles[br]
                wt = get_w(br)
                fin = []
                for k in range(MC):
                    t = finp.tile([128, PIX], MM_DT, tag="fin", name=f"fin{b}{br}{k}")
                    nc.sync.dma_start(t[:], feat_d[b, 128 * k : 128 * (k + 1), :])
                    fin.append(t)
                # k-chunk order interleaved (chunks 0,0,1,1,...) so the first
                # matmul group only waits on the first 512 KiB feature DMA.
                korder = [0, MC, 1, MC + 1, 2, MC + 2, 3, MC + 3]
                # One full-width ACT per k-chunk: (N+352)-cycle cost amortized.
                rhs = {}
                for k8 in korder:
                    r = rhsp.tile([128, PIX], MM_DT, tag="rhs", name=f"rhs{b}{br}{k8}")
                    nc.scalar.activation(
                        r[:],
                        fin[k8 % MC][:],
                        mybir.ActivationFunctionType.Relu,
                        bias=ct[:, 8 + k8 : 9 + k8],
                        scale=ct[:, k8 : k8 + 1],
                    )
                    rhs[k8] = r
                for m in range(MC):
                    ot = outp.tile([128, PIX], MM_DT, tag="ot")
                    pss = [psump.tile([128, NT], F32, tag="ps", name=f"ps{b}{br}{m}{n}") for n in range(NN)]
                    for ki, k8 in enumerate(korder):
                        w_ap = wt[:, k8, 128 * m : 128 * (m + 1)]
                        for n in range(NN):
                            nc.tensor.matmul(
                                pss[n][:],
                                w_ap,
                                rhs[k8][:, NT * n : NT * (n + 1)],
                                start=(ki == 0),
                                stop=(ki == KC - 1),
                            )
                    for n in range(NN):
                        nc.vector.tensor_copy(ot[:, NT * n : NT * (n + 1)], pss[n][:])
                    nc.gpsimd.dma_start(out_d[b, 128 * m : 128 * (m + 1), :], ot[:])
    nc.compile()
    return nc


def _fold_bn(bn):
    g, bb, m, v = [np.asarray(x, np.float32) for x in bn]
    s = g / np.sqrt(v + EPS)
    t = bb - m * s
    return s.astype(np.float32), t.astype(np.float32)


def _pack_consts(bn):
    # [128, 16]: col j in [0,8) = scale chunk j, col 8+j = bias chunk j
    s, t = _fold_bn(bn)
    out = np.zeros((128, 16), np.float32)
    out[:, 0:8] = s.reshape(8, 128).T
    out[:, 8:16] = t.reshape(8, 128).T
    return out


def kernel(frontal_features, lateral_features, params):
    global LAST_RESULTS
    f = np.ascontiguousarray(
        np.asarray(frontal_features, np.float16).reshape(B, C, PIX)
    )
    l = np.ascontiguousarray(
        np.asarray(lateral_features, np.float16).reshape(B, C, PIX)
    )
    def pack_w(w):
        # W (C out, 2C in) -> lhsT chunks [k, p, m] -> partition-major
        # [128, KC*C] so each partition's row is contiguous in DRAM.
        wt = np.asarray(w, np.float32).T.astype(np.float16)  # [2C, C]
        return np.ascontiguousarray(
            wt.reshape(KC, 128, C).transpose(1, 0, 2).reshape(128, KC * C)
        )

    wtf = pack_w(params["fnet"]["w"])
    wtl = pack_w(params["lnet"]["w"])
    cf = _pack_consts(params["fnet"]["bn"])
    cl = _pack_consts(params["lnet"]["bn"])

    if "nc" not in _CACHE:
        _CACHE["nc"] = _build_module()
    nc = _CACHE["nc"]

    in_maps = [
        {
            "f": f[BS * i : BS * (i + 1)],
            "l": l[BS * i : BS * (i + 1)],
            "wtf": wtf,
            "wtl": wtl,
            "cf": cf,
            "cl": cl,
        }
        for i in range(N_CORES)
    ]
    res = run_bass_kernel_spmd(
        nc,
        in_maps,
        core_ids=list(range(N_CORES)),
        trace=bool(int(os.environ.get("KERNEL_TRACE", "0"))),
    )
    LAST_RESULTS = res
    fout = np.concatenate([r["fo"] for r in res.results]).astype(np.float32).reshape(B, C, H, W)
    lout = np.concatenate([r["lo"] for r in res.results]).astype(np.float32).reshape(B, C, H, W)
    return fout, lout


# revision 18
# speedup vs baseline: 1.0174x; 1.0097x over previous
"""Trainium2 Bass kernel for nn_CrossSectionalAttentionFusionMLP.

Math note: the reference computes attention masks fm/lm = softmax(...) over the
last axis, then applies them via einsum('bhwv,bchw->bchw', mask, feat) — which
contracts v over the mask ONLY, i.e. multiplies feat by the softmax row-sum
(== 1).  The transfer tensors therefore equal the raw features to fp32 rounding
(~1e-7 relative, verified), so the kernel computes only

    fout = W_f @ relu(bn_f(concat([frontal, frontal])))   # (B, 512, H, W)
    lout = W_l @ relu(bn_l(concat([lateral, lateral])))

with the BatchNorm folded to per-channel scale/bias on the host.

Sharding: data-parallel over B across 8 cores (4 samples each); conv weights
and folded BN constants replicated.  Per core the 1x1 conv is a K=1024 x M=512
matmul over N=4096 pixels, done as 128x128x512 PE matmuls accumulating in PSUM,
with ScalarE producing relu(x*s+t) tiles and VectorE copying PSUM->SBUF.
"""

import os
from contextlib import ExitStack

import numpy as np

import concourse.bass as bass
import concourse.mybir as mybir
import concourse.tile as tile
from concourse import bacc
from concourse.bass_utils import run_bass_kernel_spmd

N_CORES = 8
B, C, H, W = 32, 512, 32, 32
PIX = H * W                 # 1024 pixels per sample
BS = B // N_CORES           # 4 samples per core
KC = (2 * C) // 128         # 8 contraction chunks of 128
MC = C // 128               # 4 output-channel chunks of 128
NT = 512                    # moving free dim per matmul (one PSUM bank of f32)
NN = PIX // NT              # 2 pixel chunks per sample

EPS = 1e-5

F32 = mybir.dt.float32
MM_DT = mybir.dt.float16  # full-rate PE matmul + FWL; ~3e-4 rel err at K=1024

LAST_RESULTS = None  # BassKernelResults of the most recent run (for test.py)

_CACHE = {}


def _build_module():
    nc = bacc.Bacc("TRN2", target_bir_lowering=False, debug=False)

    f_d = nc.dram_tensor("f", [BS, C, PIX], MM_DT, kind="ExternalInput").ap()
    l_d = nc.dram_tensor("l", [BS, C, PIX], MM_DT, kind="ExternalInput").ap()
    wtf_d = nc.dram_tensor("wtf", [128, KC * C], MM_DT, kind="ExternalInput").ap()
    wtl_d = nc.dram_tensor("wtl", [128, KC * C], MM_DT, kind="ExternalInput").ap()
    cf_d = nc.dram_tensor("cf", [128, 16], F32, kind="ExternalInput").ap()
    cl_d = nc.dram_tensor("cl", [128, 16], F32, kind="ExternalInput").ap()
    fo_d = nc.dram_tensor("fo", [BS, C, PIX], MM_DT, kind="ExternalOutput").ap()
    lo_d = nc.dram_tensor("lo", [BS, C, PIX], MM_DT, kind="ExternalOutput").ap()

    with tile.TileContext(nc) as tc, ExitStack() as ctx:
        wpool = ctx.enter_context(tc.tile_pool(name="weights", bufs=1))
        cpool = ctx.enter_context(tc.tile_pool(name="consts", bufs=1))
        finp = ctx.enter_context(tc.tile_pool(name="finp", bufs=10))
        rhsp = ctx.enter_context(tc.tile_pool(name="rhs", bufs=12))
        psump = ctx.enter_context(tc.tile_pool(name="psum", bufs=8, space="PSUM"))
        outp = ctx.enter_context(tc.tile_pool(name="outs", bufs=6))

        # Constants + weights go on the scalar HWDGE ring (qActDynamicHW) so
        # the sync ring (qSPDynamicHW) starts streaming features immediately;
        # output stores go on the gpsimd SWDGE ring.  Three independent
        # descriptor streams over the same 16 SDMA engines.
        c_tiles = {}
        for cname, cd in (("f", cf_d), ("l", cl_d)):
            t = cpool.tile([128, 16], F32, tag=f"c{cname}")
            nc.scalar.dma_start(t[:], cd[:, :])
            c_tiles[cname] = t
        # Replicated weights: one [128, KC, C] tile per branch (W.T chunked),
        # loaded with a single 1 MiB DMA, lazily on first use.
        w_tiles = {}

        def get_w(wname):
            if wname not in w_tiles:
                wd = wtf_d if wname == "f" else wtl_d
                t = wpool.tile([128, KC, C], MM_DT, tag=f"w{wname}")
                nc.scalar.dma_start(t[:], wd.rearrange("p (k m) -> p k m", k=KC))
                w_tiles[wname] = t
            return w_tiles[wname]

        # PE warm-up: dummy matmuls on garbage SBUF during the input DMA
        # latency so HAM un-throttles (K=8/8) before the first real matmul.
        warm = cpool.tile([128, 512], MM_DT, tag="warm")
        nc.gpsimd.memset(warm[:], 0.0)
        warm_ps = psump.tile([128, NT], F32, tag="ps", name="warmps")
        for _ in range(31):
            nc.tensor.matmul(
                warm_ps[:], warm[:, 0:128], warm[:], start=True, stop=True
            )

        for b in range(BS):
            for br, feat_d, out_d in (("f", f_d, fo_d), ("l", l_d, lo_d)):
                ct = c_tiles[br]
                wt = get_w(br)
                fin = []
                for k in range(MC):
                    t = finp.tile([128, PIX], MM_DT, tag="fin", name=f"fin{b}{br}{k}")
                    if b == 0 and br == "f" and k == 0:
                        # halve the first DMA so the first ACT/matmul start early
                        nc.sync.dma_start(t[:, 0:NT], feat_d[b, 0:128, 0:NT])
                        nc.sync.dma_start(t[:, NT:PIX], feat_d[b, 0:128, NT:PIX])
                    else:
                        nc.sync.dma_start(t[:], feat_d[b, 128 * k : 128 * (k + 1), :])
                    fin.append(t)
                # k-chunk order interleaved (chunks 0,0,1,1,...) so the first
                # matmul group only waits on the first 512 KiB feature DMA.
                korder = [0, MC, 1, MC + 1, 2, MC + 2, 3, MC + 3]
                # One full-width ACT per k-chunk: (N+352)-cycle cost amortized.
                rhs = {}
                for k8 in korder:
                    r = rhsp.tile([128, PIX], MM_DT, tag="rhs", name=f"rhs{b}{br}{k8}")
                    if b == 0 and br == "f" and k8 == 0:
                        for h in range(NN):
                            nc.scalar.activation(
                                r[:, NT * h : NT * (h + 1)],
                                fin[0][:, NT * h : NT * (h + 1)],
                                mybir.ActivationFunctionType.Relu,
                                bias=ct[:, 8 : 9],
                                scale=ct[:, 0 : 1],
                            )
                    else:
                        nc.scalar.activation(
                            r[:],
                            fin[k8 % MC][:],
                            mybir.ActivationFunctionType.Relu,
                            bias=ct[:, 8 + k8 : 9 + k8],
                            scale=ct[:, k8 : k8 + 1],
                        )
                    rhs[k8] = r
                for m in range(MC):
                    ot = outp.tile([128, PIX], MM_DT, tag="ot")
                    pss = [psump.tile([128, NT], F32, tag="ps", name=f"ps{b}{br}{m}{n}") for n in range(NN)]
                    for ki, k8 in enumerate(korder):
                        w_ap = wt[:, k8, 128 * m : 128 * (m + 1)]
                        for n in range(NN):
                            nc.tensor.matmul(
                                pss[n][:],
                                w_ap,
                                rhs[k8][:, NT * n : NT * (n + 1)],
                                start=(ki == 0),
                                stop=(ki == KC - 1),
                            )
                    for n in range(NN):
                        nc.vector.tensor_copy(ot[:, NT * n : NT * (n + 1)], pss[n][:])
                    nc.gpsimd.dma_start(out_d[b, 128 * m : 128 * (m + 1), :], ot[:])
    nc.compile()
    return nc


def _fold_bn(bn):
    g, bb, m, v = [np.asarray(x, np.float32) for x in bn]
    s = g / np.sqrt(v + EPS)
    t = bb - m * s
    return s.astype(np.float32), t.astype(np.float32)


def _pack_consts(bn):
    # [128, 16]: col j in [0,8) = scale chunk j, col 8+j = bias chunk j
    s, t = _fold_bn(bn)
    out = np.zeros((128, 16), np.float32)
    out[:, 0:8] = s.reshape(8, 128).T
    out[:, 8:16] = t.reshape(8, 128).T
    return out


def kernel(frontal_features, lateral_features, params):
    global LAST_RESULTS
    f = np.ascontiguousarray(
        np.asarray(frontal_features, np.float16).reshape(B, C, PIX)
    )
    l = np.ascontiguousarray(
        np.asarray(lateral_features, np.float16).reshape(B, C, PIX)
    )
    def pack_w(w):
        # W (C out, 2C in) -> lhsT chunks [k, p, m] -> partition-major
        # [128, KC*C] so each partition's row is contiguous in DRAM.
        wt = np.asarray(w, np.float32).T.astype(np.float16)  # [2C, C]
        return np.ascontiguousarray(
            wt.reshape(KC, 128, C).transpose(1, 0, 2).reshape(128, KC * C)
        )

    wtf = pack_w(params["fnet"]["w"])
    wtl = pack_w(params["lnet"]["w"])
    cf = _pack_consts(params["fnet"]["bn"])
    cl = _pack_consts(params["lnet"]["bn"])

    if "nc" not in _CACHE:
        _CACHE["nc"] = _build_module()
    nc = _CACHE["nc"]

    in_maps = [
        {
            "f": f[BS * i : BS * (i + 1)],
            "l": l[BS * i : BS * (i + 1)],
            "wtf": wtf,
            "wtl": wtl,
            "cf": cf,
            "cl": cl,
        }
        for i in range(N_CORES)
    ]
    res = run_bass_kernel_spmd(
        nc,
        in_maps,
        core_ids=list(range(N_CORES)),
        trace=bool(int(os.environ.get("KERNEL_TRACE", "0"))),
    )
    LAST_RESULTS = res
    fout = np.concatenate([r["fo"] for r in res.results]).astype(np.float32).reshape(B, C, H, W)
    lout = np.concatenate([r["lo"] for r in res.results]).astype(np.float32).reshape(B, C, H, W)
    return fout, lout
